# revision 6
# baseline (speedup 1.0000x reference)
"""Trainium2 kernel for nn_AdaFastFoodMergedModel.

FastFood transform: y = SCALE * Sel . H . diag(G) . Pi . H . diag(B) . x
(H = 4096-point orthonormal Walsh-Hadamard, Pi = random permutation,
Sel = row subset of size 1228).

Strategy: everything right of `x` is a fixed linear operator built from the
small inputs (B, G, Pi, row_idx), so fold it on the host into one dense
matrix W [4096, 1228] (bf16) and run y = x @ W on the TensorEngine.
Device work per core (rows sharded 8192/8 = 1024 rows):
  - W is loaded as 8 SEPARATE group tiles [128, 4, 1228] so matmuls only
    wait on the kc-group they need (group-granular dependencies), with
    accumulation emitted group-major: compute starts when group 0 lands.
  - row-tiles 0+1 are interleaved group-major across 6 psum banks so the
    PE stays fed while the W stream is still arriving; row-tiles 2..7
    then run back-to-back with W fully SBUF-resident.
  - one LDWEIGHTS (lhsT = xT k-chunk) feeds all 3 sel-chunk matmuls
    (1228 streamed columns per weight load).
  - psum evacuation split across DVE/ACT/Pool; output DMA on sync ring.
  - warm-up matmuls at t=0 ramp the PE p-state during the load phase.
No cross-core communication (data parallel over rows).
"""

import math
import sys

import numpy as np

sys.path.insert(0, "/opt/trn_rl_repo")

import ml_dtypes

ROWS, D = 8192, 4096
M = 1228
SCALE = math.sqrt(D / M)
N_CORES = 8
SHARD = ROWS // N_CORES  # 1024
P = 128
KC = D // P  # 32 contraction chunks
RT = SHARD // P  # 8 row tiles per core
SEL_CHUNKS = [(0, 512), (512, 512), (1024, 204)]  # 1228 = 512+512+204
WG = 8  # W group loads
KPG = KC // WG  # 4 kc per group
CH = 4  # column chunks per row tile
CW = D // CH  # 1024
KCC = KC // CH  # 8 k-chunks per column-chunk
WARMUP_MM = 16

# set by test harness to collect a profile
TRACE = False
LAST = {}

_CACHE = {}


def _fwht_cols(a: np.ndarray) -> np.ndarray:
    """Orthonormal FWHT along axis 0 (Sylvester/natural order)."""
    n = a.shape[0]
    x = a.copy()
    h = 1
    while h < n:
        x = x.reshape(n // (2 * h), 2, h, -1)
        lo = x[:, 0]
        hi = x[:, 1]
        x = np.stack((lo + hi, lo - hi), axis=1).reshape(n, -1)
        h *= 2
    return x * (1.0 / math.sqrt(n))


def _build_w(B, G, Pi, row_idx) -> np.ndarray:
    """W such that y = x @ W  (float32)."""
    S = np.zeros((D, M), dtype=np.float64)
    S[row_idx, np.arange(M)] = 1.0  # Sel^T
    A = _fwht_cols(S)  # H .
    A = A * G[:, None].astype(np.float64)  # diag(G) .
    A2 = np.empty_like(A)
    A2[Pi] = A  # Pi^T .
    A2 = _fwht_cols(A2)  # H .
    W = SCALE * (B[:, None].astype(np.float64) * A2)  # diag(B) .
    return W.astype(np.float32)


def _install_ntff_shim():
    """The image's antenv lacks axon_hooks; provide it so
    run_bass_kernel_spmd(trace=True) can collect an NTFF profile."""
    import types

    try:
        import antenv.axon_hooks  # noqa: F401

        return
    except ImportError:
        pass
    try:
        from trn_agent_boot.trn_boot import _ntff_profile_via_ctypes

        hook = _ntff_profile_via_ctypes("/opt/axon/libaxon_pjrt.so")
    except Exception:
        hook = None
    mod = types.ModuleType("antenv.axon_hooks")
    mod.get_axon_ntff_profile_hook = lambda: hook
    mod.set_axon_ntff_profile_hook = lambda h: None
    sys.modules["antenv.axon_hooks"] = mod


def _build_bass():
    import concourse.bass as bass
    import concourse.bacc as bacc
    import concourse.mybir as mybir
    from concourse import tile

    f32 = mybir.dt.float32
    bf16 = mybir.dt.bfloat16

    nc = bacc.Bacc("TRN2", target_bir_lowering=False, debug=False)
    x_in = nc.declare_dram_parameter("x", [SHARD, D], f32, isOutput=False)
    # W pre-arranged on host to the SBUF layout [p, kc, m] so each DMA is
    # contiguous per partition
    w_in = nc.declare_dram_parameter("w", [P, KC, M], bf16, isOutput=False)
    out = nc.declare_dram_parameter("out", [SHARD, M], f32, isOutput=True)

    with tile.TileContext(nc) as tc:
        with (
            tc.tile_pool(name="const", bufs=1) as const_pool,
            tc.tile_pool(name="xf", bufs=1) as xf_pool,
            tc.tile_pool(name="xbf", bufs=1) as xbf_pool,
            tc.tile_pool(name="xT", bufs=1) as xT_pool,
            tc.tile_pool(name="y", bufs=2) as y_pool,
            tc.tile_pool(name="psy", bufs=1, space=bass.MemorySpace.PSUM) as psy_pool,
        ):
            # --- PE warm-up: ramp the p-state while DMAs stream ---
            if WARMUP_MM:
                warm_l = const_pool.tile([P, P], bf16, tag="warm_l")
                warm_r = const_pool.tile([P, 512], bf16, tag="warm_r")
                nc.gpsimd.memset(warm_l[:], 0.0)
                nc.gpsimd.memset(warm_r[:], 0.0)
                wps = psy_pool.tile([P, 512], f32, tag="warm_ps")
                for i in range(WARMUP_MM):
                    nc.tensor.matmul(
                        wps[:],
                        warm_l[:],
                        warm_r[:],
                        start=(i == 0),
                        stop=(i == WARMUP_MM - 1),
                    )

            # --- W: 8 separate group tiles, scalar ring ---
            w_g = []
            for g in range(WG):
                wt = const_pool.tile([P, KPG, M], bf16, tag=f"w{g}")
                nc.scalar.dma_start(wt[:], w_in[:, g * KPG : (g + 1) * KPG, :])
                w_g.append(wt)

            # --- x chunk loads: gpsimd ring early; scalar joins once W is done ---
            def emit_load(rt):
                chunks = []
                for c in range(CH):
                    xfc = xf_pool.tile([P, CW], f32, tag=f"xf{rt % 4}c{c}")
                    eng = nc.gpsimd if (rt < 4 or c % 2 == 0) else nc.scalar
                    eng.dma_start(
                        xfc[:], x_in[rt * P : (rt + 1) * P, c * CW : (c + 1) * CW]
                    )
                    chunks.append(xfc)
                return chunks

            def emit_prep(rt, xfc):
                """cast chunks (DVE) + xbar transpose (sync ring)."""
                xTc = []
                for c in range(CH):
                    xtb = xbf_pool.tile([P, CW], bf16, tag=f"xtb{rt % 2}c{c}")
                    nc.vector.tensor_copy(xtb[:], xfc[c][:])
                    xT = xT_pool.tile([P, KCC, P], bf16, tag=f"xT{rt % 4}c{c}")
                    nc.sync.dma_start(xT[:], xtb[:], transpose=True)
                    xTc.append(xT)
                return xTc

            def lhs(xTc, kc):
                return xTc[kc // KCC][:, kc % KCC, :]

            def emit_evac_out(rt, psys):
                y_sb = y_pool.tile([P, M], f32, tag="y")
                nc.vector.tensor_copy(y_sb[:, 0:512], psys[0][:])
                nc.scalar.copy(y_sb[:, 512:1024], psys[1][:])
                nc.vector.tensor_copy(y_sb[:, 1024:1228], psys[2][:])
                nc.sync.dma_start(out[rt * P : (rt + 1) * P, :], y_sb[:])

            def new_psys(slot):
                return [
                    psy_pool.tile(
                        [P, sz], f32, tag=f"psy{slot}c{ci}", name=f"psy{slot}c{ci}"
                    )
                    for ci, (off, sz) in enumerate(SEL_CHUNKS)
                ]

            # --- prefetch + prep row tiles 0..3 ---
            xfs = {0: emit_load(0), 1: emit_load(1)}
            xTs = {}
            xTs[0] = emit_prep(0, xfs[0])
            xTs[1] = emit_prep(1, xfs[1])
            xfs[2] = emit_load(2)
            xfs[3] = emit_load(3)

            # --- phase 1: row tiles 0+1 interleaved, group-major ---
            ps01 = {0: new_psys(0), 1: new_psys(1)}
            for g in range(WG):
                for kc in range(g * KPG, (g + 1) * KPG):
                    for rt in (0, 1):
                        for ci, (off, sz) in enumerate(SEL_CHUNKS):
                            nc.tensor.matmul(
                                ps01[rt][ci][:],
                                lhs(xTs[rt], kc),
                                w_g[g][:, kc % KPG, off : off + sz],
                                start=(kc == 0),
                                stop=(kc == KC - 1),
                            )
            xfs[4] = emit_load(4)
            xfs[5] = emit_load(5)
            xTs[2] = emit_prep(2, xfs[2])
            xTs[3] = emit_prep(3, xfs[3])
            emit_evac_out(0, ps01[0])
            emit_evac_out(1, ps01[1])

            # --- phase 2: row tiles 2..7 back-to-back, kc-major ---
            for rt in range(2, RT):
                if rt + 4 < RT:
                    xfs[rt + 4] = emit_load(rt + 4)
                psys = new_psys(rt % 2)
                for kc in range(KC):
                    g = kc // KPG
                    for ci, (off, sz) in enumerate(SEL_CHUNKS):
                        nc.tensor.matmul(
                            psys[ci][:],
                            lhs(xTs[rt], kc),
                            w_g[g][:, kc % KPG, off : off + sz],
                            start=(kc == 0),
                            stop=(kc == KC - 1),
                        )
                if rt + 2 < RT:
                    xTs[rt + 2] = emit_prep(rt + 2, xfs[rt + 2])
                emit_evac_out(rt, psys)

    nc.compile()
    return nc


def kernel(x, B, G, Pi, row_idx):
    x = np.ascontiguousarray(np.asarray(x, dtype=np.float32))
    B = np.asarray(B, dtype=np.float32)
    G = np.asarray(G, dtype=np.float32)
    Pi = np.asarray(Pi, dtype=np.int32)
    row_idx = np.asarray(row_idx, dtype=np.int32)

    W = _build_w(B, G, Pi, row_idx).astype(ml_dtypes.bfloat16)
    # rearrange to SBUF layout [p, kc, m]: W[kc*128+p, m] -> Wp[p, kc, m]
    Wp = np.ascontiguousarray(W.reshape(KC, P, M).transpose(1, 0, 2))

    if "nc" not in _CACHE:
        _CACHE["nc"] = _build_bass()
    nc = _CACHE["nc"]

    if TRACE:
        _install_ntff_shim()

    from concourse.bass_utils import run_bass_kernel_spmd

    shards = [x[i * SHARD : (i + 1) * SHARD] for i in range(N_CORES)]
    in_maps = [{"x": shards[i], "w": Wp} for i in range(N_CORES)]

    res = run_bass_kernel_spmd(
        nc, in_maps, core_ids=list(range(N_CORES)), trace=TRACE
    )
    LAST["exec_time_ns"] = getattr(res, "exec_time_ns", None)
    LAST["results"] = res

    outs = [np.asarray(res.results[i]["out"]) for i in range(N_CORES)]
    return np.concatenate(outs, axis=0).astype(np.float32)


if __name__ == "__main__":
    rng = np.random.default_rng(0)
    x = rng.standard_normal((ROWS, D), dtype=np.float32)
    B = (rng.integers(0, 2, D) * 2 - 1).astype(np.float32)
    G = rng.standard_normal(D, dtype=np.float32)
    Pi = rng.permutation(D).astype(np.int32)
    row_idx = rng.permutation(D)[:M].astype(np.int32)
    y = kernel(x=x, B=B, G=G, Pi=Pi, row_idx=row_idx)
    print("out", y.shape, y.dtype)
